# revision 1
# baseline (speedup 1.0000x reference)
"""TRN2 Bass kernel for 16-head causal MHA (B=4, T=2048, C=2048), fp32.

Sharding: 8 cores = 4 batches x 2 head-groups (8 heads each).  Each core
computes q/k/v projections for its head group on its batch (tensor-parallel
column split of Wq/Wk/Wv), causal flash-style attention in the S^T layout
(scores computed transposed so softmax normalization is a partition-dim
reduction done with a ones-matmul), and a partial output projection with the
row slice of Wp.  The two head-group partials per batch are summed on the
host (the "all-reduce after proj" step), plus the output bias.

All matmuls run in float32r (full PE rate at N>=256, fp32-equivalent accuracy
as measured on HW).  Softmax uses exp without max-subtraction (scores are
O(+-10) for this problem's 0.02-scaled weights; exp is computed in fp32 from
the fp32 PSUM scores, so there is no overflow risk), with the causal mask
applied additively (-1e10) on diagonal blocks before the exp, and strictly
above-diagonal blocks skipped entirely.
"""
import math
import os

import numpy as np

import concourse.bass as bass
import concourse.tile as tile
from concourse import bacc, mybir
from concourse.bass_utils import run_bass_kernel_spmd

f32 = mybir.dt.float32
f32r = mybir.dt.float32r
AF = mybir.ActivationFunctionType

N_CORES = 8
HD = 128                      # head dim
NEG = -1e10                   # additive causal mask value

# results of the last run_bass_kernel_spmd call (for test harness profiling)
LAST_RESULT = None


def build_nc(T=2048, E=2048, D=1024, NOD=2048, TG=512, bias=False, num_devices=N_CORES,
             phases=("ab", "c", "d"), cparts=("mask", "dsum", "scat", "bcast", "mul")):
    """Build + compile the per-core Bass program.

    T: sequence length; E: embedding (contraction) dim; D: this core's head
    slice width (NH = D/128 heads); NOD: output projection width; TG: q-group
    width for attention; bias: if True, inputs carry one extra 128-row chunk
    holding [bias; zeros] against an xT with a ones row.
    """
    NH = D // HD
    EC = E // 128 + (1 if bias else 0)
    Ep = EC * 128
    TC = T // 128            # 128-row tiles along T
    TGC = T // TG            # q groups
    NMASK = TG // 128        # diagonal mask variants
    ODG = NOD // 512
    scale = 1.0 / math.sqrt(HD)
    VDG = 256                # v-phase d-group width

    nc = bacc.Bacc("TRN2", target_bir_lowering=False, debug=False,
                   num_devices=num_devices)

    xT = nc.dram_tensor("xT", [Ep, T], f32r, kind="ExternalInput")
    wq = nc.dram_tensor("wq", [Ep, D], f32r, kind="ExternalInput")
    wk = nc.dram_tensor("wk", [Ep, D], f32r, kind="ExternalInput")
    wv = nc.dram_tensor("wv", [Ep, D], f32r, kind="ExternalInput")
    wp = nc.dram_tensor("wp", [D, NOD], f32r, kind="ExternalInput")
    ones_d = nc.dram_tensor("ones", [128, 1], f32r, kind="ExternalInput")
    masks_d = nc.dram_tensor("masks", [NMASK, 128, TG], f32, kind="ExternalInput")
    y_d = nc.dram_tensor("y", [T, NOD], f32, kind="ExternalOutput")

    qt_sp = nc.dram_tensor("qt_sp", [D, T], f32r, kind="Internal")
    kt_sp = nc.dram_tensor("kt_sp", [D, T], f32r, kind="Internal")
    VN = min(512, D)
    NVS = D // VN
    v_sps = [nc.dram_tensor(f"v_sp{i}", [T, VN], f32r, kind="Internal")
             for i in range(NVS)]
    dscr = nc.dram_tensor("dscr", [NH * TGC, TG], f32, kind="Internal")
    dscr_ap = dscr.ap()

    with tile.TileContext(nc) as tc:
        # ---------------- phase A+B: q/k/v projections ----------------
        with (
            tc.tile_pool(name="xt", bufs=1) as xt_pool,
            tc.tile_pool(name="ab_stage", bufs=4) as ab_stage,
            tc.tile_pool(name="ab_psum", bufs=8, space="PSUM") as ab_psum,
        ):
            xt_sb = xt_pool.tile([128, EC * T], f32r)

            def xt_e(e):
                return xt_sb[:, e * T:(e + 1) * T]

            spills = (qt_sp, kt_sp)
            wds = (wq, wk)
            wpairs = [(w_i, dc) for w_i in range(2) for dc in range(D // 128)]

            wv_pool0_cm = tc.tile_pool(name="wv0", bufs=1)
            wv_pool0 = wv_pool0_cm.__enter__()
            wvgs = {}

            def load_wvg(pool, dg):
                wvg = pool.tile([128, EC * VN], f32r, tag=f"wvg{dg}",
                                name=f"wvg_{dg}")
                nc.sync.dma_start(
                    wvg.rearrange("p (ec d) -> p ec d", ec=EC),
                    wv.rearrange("(ec p) d -> p ec d", p=128)[
                        :, :, dg * VN:(dg + 1) * VN],
                )
                wvgs[dg] = wvg

            with tc.tile_pool(name="wcola", bufs=3) as wcol_pool:
                def load_wcol(w_i, dc):
                    wcol = wcol_pool.tile([128, EC * 128], f32r, tag="wcol",
                                          name=f"wcol_{w_i}_{dc}")
                    nc.sync.dma_start(
                        wcol.rearrange("p (ec d) -> p ec d", ec=EC),
                        wds[w_i].rearrange("(ec p) d -> p ec d", p=128)[
                            :, :, dc * 128:(dc + 1) * 128],
                    )
                    return wcol

                # preload the first two weight columns BEFORE the xT chunks
                # so the first wave's matmuls start as soon as xT chunk 0
                # lands rather than after the whole 16MB input load.
                wcol_q = [load_wcol(*wpairs[0])]
                nc.sync.dma_start(xt_sb[:, 0:T], xT[0:128, :])
                wcol_q.append(load_wcol(*wpairs[1]))
                for e in range(1, EC):
                    nc.sync.dma_start(
                        xt_sb[:, e * T:(e + 1) * T],
                        xT[e * 128:(e + 1) * 128, :],
                    )
                load_wvg(wv_pool0, 0)

                # Q^T / K^T: one (weight, d-chunk) per wave of 4 PSUM groups,
                # e-major inside the wave; 8 banks = two waves in flight.
                for wi, (w_i, dc) in enumerate(wpairs):
                    wcol = wcol_q.pop(0)
                    if wi + 2 < len(wpairs):
                        wcol_q.append(load_wcol(*wpairs[wi + 2]))
                    ngrp = T // TG
                    pss = [ab_psum.tile([128, TG], f32, tag="abps",
                                        name=f"abps_{w_i}_{dc}_{tg}")
                           for tg in range(ngrp)]
                    for e in range(EC):
                        for tg in range(ngrp):
                            nc.tensor.matmul(
                                pss[tg][:],
                                wcol[:, e * 128:(e + 1) * 128],
                                xt_e(e)[:, tg * TG:(tg + 1) * TG],
                                start=(e == 0), stop=(e == EC - 1),
                            )
                    for tg in range(ngrp):
                        st = ab_stage.tile([128, TG], f32r, tag="abst")
                        nc.scalar.copy(st[:], pss[tg][:])
                        nc.sync.dma_start(
                            spills[w_i][dc * 128:(dc + 1) * 128,
                                        tg * TG:(tg + 1) * TG],
                            st[:],
                        )

            # V in natural [t, d] layout at full N=512 moving width.
            # wvg0 was prefetched during phase A; load the rest here (their
            # pool reuses the space the wcol pool released).
            with tc.tile_pool(name="wvrest", bufs=1) as wv_pool1:
                for dg in range(1, NVS):
                    load_wvg(wv_pool1, dg)
                for dg in range(NVS):
                        wvg = wvgs[dg]
                        for tt in range(TC):
                            ps = ab_psum.tile([128, VN], f32, tag="abps",
                                              name=f"vps_{dg}_{tt}")
                            for e in range(EC):
                                nc.tensor.matmul(
                                    ps[:],
                                    xt_e(e)[:, tt * 128:(tt + 1) * 128],
                                    wvg[:, e * VN:(e + 1) * VN],
                                    start=(e == 0), stop=(e == EC - 1),
                                )
                            st = ab_stage.tile([128, VN], f32r, tag="abst")
                            nc.scalar.copy(st[:], ps[:])
                            nc.sync.dma_start(
                                v_sps[dg][tt * 128:(tt + 1) * 128, :],
                                st[:],
                            )
            wv_pool0_cm.__exit__(None, None, None)

        # ---------------- phase C: attention ----------------
        with tc.tile_pool(name="atn", bufs=1) as atn_pool, \
                tc.tile_pool(name="dwork", bufs=2) as d_pool:
            atn_all = atn_pool.tile([128, NH * T], f32r)

            def load_wpog(og):
                wpog = d_pool.tile([128, NH * 512], f32r, tag="wpog",
                                   name=f"wpog_{og}")
                nc.sync.dma_start(
                    wpog.rearrange("p (dc o) -> p dc o", dc=NH),
                    wp.rearrange("(dc p) o -> p dc o", p=128)[
                        :, :, og * 512:(og + 1) * 512],
                )
                return wpog

            # prefetch the first Wp column group during attention
            wpog_q = [load_wpog(0)] if ("d" in phases and ODG) else []
            with (
                tc.tile_pool(name="heads", bufs=2) as h_pool,
                tc.tile_pool(name="cwork", bufs=4) as c_pool,
                tc.tile_pool(name="consts", bufs=1) as const_pool,
                tc.tile_pool(name="c_psum_s", bufs=3, space="PSUM") as c_psum_s,
                tc.tile_pool(name="c_psum_a", bufs=3, space="PSUM") as c_psum_a,
                tc.tile_pool(name="c_psum_d", bufs=2, space="PSUM") as c_psum_d,
            ):
                ones_sb = const_pool.tile([128, 1], f32r)
                nc.sync.dma_start(ones_sb[:], ones_d[:])
                masks_sb = const_pool.tile([128, NMASK * TG], f32)
                nc.sync.dma_start(
                    masks_sb.rearrange("p (j q) -> p j q", j=NMASK),
                    masks_d.rearrange("j p q -> p j q"),
                )

                pending_norm = []

                def emit_norm(h, qg, slot, atn_u, dsum_ps):
                    # deferred one group AND priority-pushed later so the
                    # 3.3us DVE reciprocal never delays the next group's
                    # mask adds in the DVE stream
                    recip = c_pool.tile([1, TG], f32, tag="recip",
                                        name=f"recip_{slot}")
                    with tc.high_priority(offset=-300):
                        nc.vector.reciprocal(recip[:], dsum_ps[:])
                    nc.sync.dma_start(dscr[slot:slot + 1, :], recip[:])
                    recipB = c_pool.tile([128, TG], f32, tag="recipB",
                                         name=f"recipB_{slot}")
                    nc.gpsimd.dma_start(
                        out=recipB[:],
                        in_=bass.AP(tensor=dscr_ap.tensor, offset=slot * TG,
                                    ap=[[0, 128], [1, TG]]),
                    )
                    nc.gpsimd.tensor_mul(
                        atn_all[:, h * T + qg * TG:h * T + (qg + 1) * TG],
                        atn_u[:], recipB[:])

                for h in range(NH if "c" in phases else 0):
                    qt_h = h_pool.tile([128, T], f32r, tag="qt")
                    nc.sync.dma_start(qt_h[:], qt_sp[h * 128:(h + 1) * 128, :])
                    kt_h = h_pool.tile([128, T], f32r, tag="kt")
                    nc.sync.dma_start(kt_h[:], kt_sp[h * 128:(h + 1) * 128, :])
                    v_h = h_pool.tile([128, T], f32r, tag="vh")
                    hv, hcol = divmod(h * 128, VN)
                    nc.sync.dma_start(
                        v_h.rearrange("p (tc d) -> p tc d", d=128),
                        v_sps[hv].rearrange("(tc p) d -> p tc d", p=128)[
                            :, :, hcol:hcol + 128],
                    )
                    for qg in range(TGC):
                        nk = (qg + 1) * NMASK
                        atn_ps = c_psum_a.tile([128, TG], f32, tag="atnps")
                        dsum_ps = c_psum_d.tile([1, TG], f32, tag="dsum")
                        for kc in range(nk):
                            s_ps = c_psum_s.tile([128, TG], f32, tag="sps")
                            nc.tensor.matmul(
                                s_ps[:],
                                kt_h[:, kc * 128:(kc + 1) * 128],
                                qt_h[:, qg * TG:(qg + 1) * TG],
                                start=True, stop=True,
                            )
                            j = kc - qg * NMASK
                            if j >= 0 and "mask" in cparts:
                                nc.vector.tensor_add(
                                    s_ps[:], s_ps[:],
                                    masks_sb[:, j * TG:(j + 1) * TG])
                            p_t = c_pool.tile([128, TG], f32r, tag="pt")
                            nc.scalar.activation(p_t[:], s_ps[:], AF.Exp, scale=scale)
                            nc.tensor.matmul(
                                atn_ps[:],
                                v_h[:, kc * 128:(kc + 1) * 128],
                                p_t[:],
                                start=(kc == 0), stop=(kc == nk - 1),
                            )
                            nc.tensor.matmul(
                                dsum_ps[:], ones_sb[:], p_t[:],
                                start=(kc == 0), stop=(kc == nk - 1),
                            )
                        # denominator: partition-sum via ones-matmul, then
                        # reciprocal row scattered/broadcast via DRAM roundtrip
                        slot = h * TGC + qg
                        atn_u = c_pool.tile([128, TG], f32, tag="atnu")
                        nc.scalar.copy(atn_u[:], atn_ps[:])
                        pending_norm.append((h, qg, slot, atn_u, dsum_ps))
                        if len(pending_norm) > 1:
                            emit_norm(*pending_norm.pop(0))
                for args in pending_norm:
                    emit_norm(*args)
                pending_norm.clear()

            # ---------------- phase D: output projection ----------------
            with (
                tc.tile_pool(name="d_stage", bufs=4) as d_stage,
                tc.tile_pool(name="d_psum", bufs=3, space="PSUM") as d_psum,
            ):
                for og in range(ODG if "d" in phases else 0):
                    wpog = wpog_q.pop(0)
                    if og + 1 < ODG:
                        wpog_q.append(load_wpog(og + 1))
                    for tt in range(TC):
                        ps = d_psum.tile([128, 512], f32, tag="yps")
                        for hc in range(NH):
                            nc.tensor.matmul(
                                ps[:],
                                atn_all[:, hc * T + tt * 128:hc * T + (tt + 1) * 128],
                                wpog[:, hc * 512:(hc + 1) * 512],
                                start=(hc == 0), stop=(hc == NH - 1),
                            )
                        st = d_stage.tile([128, 512], f32, tag="yst")
                        nc.scalar.copy(st[:], ps[:])
                        nc.sync.dma_start(
                            y_d[tt * 128:(tt + 1) * 128, og * 512:(og + 1) * 512],
                            st[:],
                        )

    nc.compile()
    return nc


def _make_masks(TG):
    """masks[j][kk, qq] = 0 where kk <= qq - 128*j else NEG."""
    NMASK = TG // 128
    kk = np.arange(128)[:, None]
    qq = np.arange(TG)[None, :]
    return np.stack(
        [np.where(kk <= qq - 128 * j, 0.0, NEG) for j in range(NMASK)]
    ).astype(np.float32)


def _augment(mat_t, bias_row, pad_to):
    """Append [bias_row; zeros] below mat_t so it has pad_to rows."""
    extra = np.zeros((pad_to - mat_t.shape[0], mat_t.shape[1]), np.float32)
    extra[0] = bias_row
    return np.concatenate([mat_t, extra], axis=0)


_NC_CACHE = {}


def _get_nc(bias):
    key = bias
    if key not in _NC_CACHE:
        _NC_CACHE[key] = build_nc(bias=bias)
    return _NC_CACHE[key]


def kernel(x, Wq, bq, Wk, bk, Wv, bv, Wp, bp):
    global LAST_RESULT
    x = np.ascontiguousarray(np.asarray(x, np.float32))
    Wq, bq = np.asarray(Wq, np.float32), np.asarray(bq, np.float32)
    Wk, bk = np.asarray(Wk, np.float32), np.asarray(bk, np.float32)
    Wv, bv = np.asarray(Wv, np.float32), np.asarray(bv, np.float32)
    Wp, bp = np.asarray(Wp, np.float32), np.asarray(bp, np.float32)

    B, T, C = x.shape
    assert (B, T, C) == (4, 2048, 2048), (B, T, C)
    D = 1024  # head-group width: 8 heads per core
    bias = bool(np.any(bq) or np.any(bk) or np.any(bv))
    nc = _get_nc(bias)

    masks = _make_masks(512)
    ones = np.ones((128, 1), np.float32)
    Ep = C + 128 if bias else C

    in_maps = []
    for c in range(N_CORES):
        b, g = c // 2, c % 2
        xt = x[b].T
        wq_g = Wq[:, g * D:(g + 1) * D]
        wk_g = Wk[:, g * D:(g + 1) * D]
        wv_g = Wv[:, g * D:(g + 1) * D]
        if bias:
            xt = _augment(xt, np.ones(T, np.float32), Ep)
            wq_g = _augment(wq_g, bq[g * D:(g + 1) * D], Ep)
            wk_g = _augment(wk_g, bk[g * D:(g + 1) * D], Ep)
            wv_g = _augment(wv_g, bv[g * D:(g + 1) * D], Ep)
        in_maps.append({
            "xT": np.ascontiguousarray(xt),
            "wq": np.ascontiguousarray(wq_g),
            "wk": np.ascontiguousarray(wk_g),
            "wv": np.ascontiguousarray(wv_g),
            "wp": np.ascontiguousarray(Wp[g * D:(g + 1) * D, :]),
            "ones": ones,
            "masks": masks,
        })

    trace = bool(os.environ.get("MHA_TRACE"))
    res = run_bass_kernel_spmd(nc, in_maps, core_ids=list(range(N_CORES)),
                               trace=trace)
    LAST_RESULT = res

    out = np.empty((B, T, C), np.float32)
    for b in range(B):
        out[b] = res.results[2 * b]["y"] + res.results[2 * b + 1]["y"]
    out += bp[None, None, :]
    return out



# revision 4
# speedup vs baseline: 1.1205x; 1.1205x over previous
"""TRN2 Bass kernel for 16-head causal MHA (B=4, T=2048, C=2048), fp32 in/out.

Sharding: 8 cores = 4 batches x 2 head-groups (8 heads each).  Each core
computes q/k/v projections for its head group on its batch (tensor-parallel
column split of Wq/Wk/Wv), causal attention in the S^T layout, and a partial
output projection with the row slice of Wp.  The two head-group partials per
batch are summed on the host, plus the output bias.

Design (v2, bf16 datapath):
- All operand data (x^T, Wq/Wk/Wv/Wp, q/k/v, p, atn) is bf16; every matmul
  accumulates in fp32 PSUM.  bf16 stationaries get FWL (fast weight load,
  ~53ns/128-col) so LDWEIGHTS hides fully under the 512-col moving pass.
- q^T/k^T ([d,t] layout) and v ([t,d] layout) stay resident in SBUF (12 MB)
  -- no DRAM spill roundtrips between projection and attention.
- Softmax in the S^T layout: scores^T [k,q] per 128-k-chunk; exp on the
  Scalar engine batched over two PSUM banks ([128,1024]) to amortize the
  ~352-cycle ACTIVATE overhead; causal masking is multiplicative (one
  [128,128] lower-triangle bf16 tile on DVE, only on the 4 diagonal 128x128
  sub-blocks per (head, q-group)); strictly-above-diagonal work is skipped,
  and the diagonal blocks are column-trimmed (packed variable-width scores /
  attn / dsum matmuls).
- Softmax denominator via ones-matmul ([1,512]-out, accumulated in PSUM over
  k-chunks); reciprocal is reshaped through a DRAM roundtrip to [128,4] so
  the DVE reciprocal takes ~85ns instead of 3.3us on one lane; the recip row
  is broadcast back via a partition-stride-0 DMA read and applied by GPSIMD.
- Output projection (Wp row-slice) is interleaved per q-group into the
  attention stream one q-group behind, so its PE work fills attention's
  ACT-bound stretches; y tiles DMA out on the Vector queue as they finish.
"""
import math
import os
from collections import deque

import ml_dtypes
import numpy as np

import concourse.bass as bass
import concourse.tile as tile
from concourse import bacc, mybir
from concourse.bass_utils import run_bass_kernel_spmd

f32 = mybir.dt.float32
bf16 = mybir.dt.bfloat16
AF = mybir.ActivationFunctionType
BF = ml_dtypes.bfloat16

N_CORES = 8
HD = 128                      # head dim

# results of the last run_bass_kernel_spmd call (for test harness profiling)
LAST_RESULT = None


def build_nc(T=2048, E=2048, D=1024, NOD=2048, TG=512, bias=False,
             num_devices=N_CORES):
    """Build + compile the per-core Bass program.

    T: sequence length; E: embedding (contraction) dim; D: this core's head
    slice width (NH = D/128 heads); NOD: output projection width; TG: q-group
    width for attention; bias: if True, inputs carry one extra 128-row chunk
    holding [bias; zeros] against an xT with a ones row.
    """
    NH = D // HD              # heads per core
    EC = E // 128 + (1 if bias else 0)
    TC = T // 128             # 128-row tiles along T
    TGC = T // TG             # q-groups
    ODG = NOD // 512          # out-proj column groups
    VN = 512                  # v-projection moving width
    NVS = D // VN
    NSLOT = NH * TGC
    scale = 1.0 / math.sqrt(HD)

    nc = bacc.Bacc("TRN2", target_bir_lowering=False, debug=False,
                   num_devices=num_devices)

    xT_d = nc.dram_tensor("xT", [EC * 128, T], bf16, kind="ExternalInput")
    wq_d = nc.dram_tensor("wq", [EC * 128, D], bf16, kind="ExternalInput")
    wk_d = nc.dram_tensor("wk", [EC * 128, D], bf16, kind="ExternalInput")
    wv_d = nc.dram_tensor("wv", [EC * 128, D], bf16, kind="ExternalInput")
    wp_d = nc.dram_tensor("wp", [D, NOD], bf16, kind="ExternalInput")
    tri_d = nc.dram_tensor("tri", [128, 128], bf16, kind="ExternalInput")
    ones_d = nc.dram_tensor("ones", [128, 1], bf16, kind="ExternalInput")
    y_d = nc.dram_tensor("y", [T, NOD], f32, kind="ExternalOutput")

    dsraw = nc.dram_tensor("dsraw", [NSLOT, TG], f32, kind="Internal")
    dsrec = nc.dram_tensor("dsrec", [NSLOT, TG], f32, kind="Internal")
    dsraw_ap = dsraw.ap()
    dsrec_ap = dsrec.ap()

    with tile.TileContext(nc) as tc:
        with tc.tile_pool(name="persist", bufs=1) as persist:
            # q^T/k^T in [d, t] layout (head h = 128-row chunk h), v in
            # natural [t, d] layout ([t%128, tt*D + d]).
            qt_all = persist.tile([128, NH * T], bf16)
            kt_all = persist.tile([128, NH * T], bf16)
            v_all = persist.tile([128, TC * D], bf16)
            ones_sb = persist.tile([128, 1], bf16)
            tri_sb = persist.tile([128, 128], bf16)
            scr = persist.tile([1, 1], f32)
            nc.sync.dma_start(ones_sb[:], ones_d[:])
            nc.sync.dma_start(tri_sb[:], tri_d[:])
            # dummy exp: pulls the ~2.7us ACT table load off phase C's
            # critical path (runs during the projection phase).
            nc.scalar.activation(scr[:], ones_sb[0:1, 0:1], AF.Exp, scale=1.0)

            # ---------------- phase A+B: q/k/v projections ----------------
            with (
                tc.tile_pool(name="xt", bufs=1) as xt_pool,
                tc.tile_pool(name="wcola", bufs=3) as wcol_pool,
                tc.tile_pool(name="wvp", bufs=1) as wv_pool,
                tc.tile_pool(name="ab_psum", bufs=8, space="PSUM") as ab_psum,
            ):
                xt_sb = xt_pool.tile([128, EC * T], bf16)

                def xt_e(e):
                    return xt_sb[:, e * T:(e + 1) * T]

                dsts = (qt_all, kt_all)
                wds = (wq_d, wk_d)
                wpairs = [(w_i, dc) for w_i in range(2) for dc in range(D // 128)]

                def load_wcol(w_i, dc):
                    wcol = wcol_pool.tile([128, EC * 128], bf16, tag="wcol",
                                          name=f"wcol_{w_i}_{dc}")
                    nc.sync.dma_start(
                        wcol.rearrange("p (ec d) -> p ec d", ec=EC),
                        wds[w_i].rearrange("(ec p) d -> p ec d", p=128)[
                            :, :, dc * 128:(dc + 1) * 128],
                    )
                    return wcol

                # first two weight columns BEFORE the xT chunks so the first
                # wave's matmuls start as soon as xT chunk 0 lands.
                wcol_q = [load_wcol(*wpairs[0])]
                nc.sync.dma_start(xt_sb[:, 0:T], xT_d[0:128, :])
                wcol_q.append(load_wcol(*wpairs[1]))
                for e in range(1, EC):
                    nc.sync.dma_start(
                        xt_sb[:, e * T:(e + 1) * T],
                        xT_d[e * 128:(e + 1) * 128, :],
                    )
                wvgs = []
                for dg in range(NVS):
                    wvg = wv_pool.tile([128, EC * VN], bf16, tag=f"wvg{dg}",
                                       name=f"wvg_{dg}")
                    nc.sync.dma_start(
                        wvg.rearrange("p (ec d) -> p ec d", ec=EC),
                        wv_d.rearrange("(ec p) d -> p ec d", p=128)[
                            :, :, dg * VN:(dg + 1) * VN],
                    )
                    wvgs.append(wvg)

                # Q^T / K^T: one (weight, d-chunk) per wave of 4 PSUM banks,
                # e-major inside the wave; 8 banks = two waves in flight.
                # PSUM results copy straight into the resident qt/kt tiles.
                for wi, (w_i, dc) in enumerate(wpairs):
                    wcol = wcol_q.pop(0)
                    if wi + 2 < len(wpairs):
                        wcol_q.append(load_wcol(*wpairs[wi + 2]))
                    ngrp = T // TG
                    pss = [ab_psum.tile([128, TG], f32, tag="abps",
                                        name=f"abps_{w_i}_{dc}_{tg}")
                           for tg in range(ngrp)]
                    for e in range(EC):
                        for tg in range(ngrp):
                            nc.tensor.matmul(
                                pss[tg][:],
                                wcol[:, e * 128:(e + 1) * 128],
                                xt_e(e)[:, tg * TG:(tg + 1) * TG],
                                start=(e == 0), stop=(e == EC - 1),
                            )
                    for tg in range(ngrp):
                        nc.scalar.copy(
                            dsts[w_i][:, dc * T + tg * TG:dc * T + (tg + 1) * TG],
                            pss[tg][:])

                # V in natural [t, d] layout at full 512 moving width.
                for tt in range(TC):
                    for dg in range(NVS):
                        ps = ab_psum.tile([128, VN], f32, tag="abps",
                                          name=f"vps_{tt}_{dg}")
                        for e in range(EC):
                            nc.tensor.matmul(
                                ps[:],
                                xt_e(e)[:, tt * 128:(tt + 1) * 128],
                                wvgs[dg][:, e * VN:(e + 1) * VN],
                                start=(e == 0), stop=(e == EC - 1),
                            )
                        nc.scalar.copy(
                            v_all[:, tt * D + dg * VN:tt * D + (dg + 1) * VN],
                            ps[:])

            # ---------------- phase C+D: attention + out-proj ----------------
            with (
                tc.tile_pool(name="cd", bufs=1) as cd_pool,
                tc.tile_pool(name="pt", bufs=3) as pt_pool,
                tc.tile_pool(name="sm", bufs=2) as sm_pool,
                tc.tile_pool(name="s_psum", bufs=2, space="PSUM") as s_psum,
                tc.tile_pool(name="a_psum", bufs=1, space="PSUM") as a_psum,
                tc.tile_pool(name="d_psum", bufs=1, space="PSUM") as d_psum,
                tc.tile_pool(name="y_psum", bufs=2, space="PSUM") as y_psum,
            ):
                atn_all = cd_pool.tile([128, NH * T], bf16)
                wp_sb = cd_pool.tile([128, NH * ODG * 512], bf16)
                # wp load issued on the Scalar queue so the Sync queue stays
                # dedicated to the low-latency reciprocal DMA chains.
                nc.scalar.dma_start(
                    wp_sb.rearrange("p (hc og o) -> p hc og o", hc=NH, og=ODG),
                    wp_d.rearrange("(hc p) (og o) -> p hc og o", p=128, o=512),
                )
                dsum_t = d_psum.tile([1, TG], f32)

                def emit_cgroup(qg, h):
                    qbase = qg * TG
                    npairs = 2 * (qg + 1)
                    nk = 4 * (qg + 1)
                    kc0 = qg * 4           # first diagonal k-chunk

                    def pair_desc(p):
                        # [(kc, soff, width, qoff)], exp width
                        if p == npairs - 2:
                            return [(kc0, 0, 512, 0),
                                    (kc0 + 1, 512, 384, 128)], 896
                        if p == npairs - 1:
                            return [(kc0 + 2, 0, 256, 256),
                                    (kc0 + 3, 256, 128, 384)], 384
                        return [(2 * p, 0, 512, 0),
                                (2 * p + 1, 512, 512, 0)], 1024

                    pts = [None] * npairs

                    def emit_av(p):
                        # attn + dsum matmuls for pair p (after its exp/mask)
                        parts, _ = pair_desc(p)
                        p_t = pts[p]
                        for (kc, soff, w, qoff) in parts:
                            nc.tensor.matmul(
                                atn_ps[:, qoff:qoff + w],
                                v_all[:, kc * D + h * HD:kc * D + (h + 1) * HD],
                                p_t[:, soff:soff + w],
                                start=(kc == 0), stop=(kc == nk - 1),
                            )
                        for (kc, soff, w, qoff) in parts:
                            nc.tensor.matmul(
                                dsum_t[0:1, qoff:qoff + w],
                                ones_sb[:],
                                p_t[:, soff:soff + w],
                                start=(kc == 0), stop=(kc == nk - 1),
                            )

                    atn_ps = a_psum.tile([128, TG], f32, tag="atn",
                                         name=f"atn_{qg}_{h}")
                    for p in range(npairs):
                        parts, expw = pair_desc(p)
                        s_pair = s_psum.tile([128, 2 * TG], f32, tag="sp",
                                             name=f"sp_{qg}_{h}_{p}")
                        # pair B packs both score blocks into one PSUM bank:
                        # exactly one start (bank pending-zero mark) and one
                        # stop per bank, with the second block overwriting
                        # its own (still-pending) byte range.
                        packed = p == npairs - 1
                        for pi, (kc, soff, w, qoff) in enumerate(parts):
                            nc.tensor.matmul(
                                s_pair[:, soff:soff + w],
                                kt_all[:, h * T + kc * 128:h * T + (kc + 1) * 128],
                                qt_all[:, h * T + qbase + qoff:h * T + qbase + 512],
                                start=(not packed or pi == 0),
                                stop=(not packed or pi == len(parts) - 1),
                            )
                        p_t = pt_pool.tile([128, 2 * TG], bf16, tag="pt",
                                           name=f"pt_{qg}_{h}_{p}")
                        pts[p] = p_t
                        nc.scalar.activation(p_t[:, 0:expw], s_pair[:, 0:expw],
                                             AF.Exp, scale=scale)
                        if p >= npairs - 2:
                            # multiplicative causal mask on the two 128-wide
                            # partial-triangle sections of this pair
                            for (kc, soff, w, qoff) in parts:
                                nc.vector.tensor_mul(
                                    p_t[:, soff:soff + 128],
                                    p_t[:, soff:soff + 128],
                                    tri_sb[:])
                        if p > 0:
                            emit_av(p - 1)
                    emit_av(npairs - 1)

                    # normalization: dsum -> DRAM -> [128,4] recip -> DRAM ->
                    # [128,TG] broadcast -> gpsimd multiply into atn_all
                    slot = qg * NH + h
                    atn_u = sm_pool.tile([128, TG], f32, tag="atnu",
                                         name=f"atnu_{slot}")
                    nc.vector.tensor_copy(atn_u[:], atn_ps[:])
                    ds_sb = sm_pool.tile([1, TG], f32, tag="dssb",
                                         name=f"dssb_{slot}")
                    nc.vector.tensor_copy(ds_sb[:], dsum_t[:])
                    nc.sync.dma_start(dsraw[slot:slot + 1, :], ds_sb[:])
                    dsr = sm_pool.tile([128, 4], f32, tag="dsr",
                                       name=f"dsr_{slot}")
                    nc.sync.dma_start(
                        dsr[:],
                        bass.AP(tensor=dsraw_ap.tensor, offset=slot * TG,
                                ap=[[4, 128], [1, 4]]))
                    rr = sm_pool.tile([128, 4], f32, tag="rr",
                                      name=f"rr_{slot}")
                    nc.vector.reciprocal(rr[:], dsr[:])
                    nc.sync.dma_start(
                        bass.AP(tensor=dsrec_ap.tensor, offset=slot * TG,
                                ap=[[4, 128], [1, 4]]),
                        rr[:])
                    recipB = sm_pool.tile([128, TG], f32, tag="rB",
                                          name=f"rB_{slot}")
                    nc.gpsimd.dma_start(
                        out=recipB[:],
                        in_=bass.AP(tensor=dsrec_ap.tensor, offset=slot * TG,
                                    ap=[[0, 128], [1, TG]]))
                    nc.gpsimd.tensor_mul(
                        atn_all[:, h * T + qbase:h * T + qbase + TG],
                        atn_u[:], recipB[:])

                def emit_dblock(tt, og):
                    ps = y_psum.tile([128, 512], f32, tag="yps",
                                     name=f"yps_{tt}_{og}")
                    for hc in range(NH):
                        nc.tensor.matmul(
                            ps[:],
                            atn_all[:, hc * T + tt * 128:hc * T + (tt + 1) * 128],
                            wp_sb[:, (hc * ODG + og) * 512:(hc * ODG + og + 1) * 512],
                            start=(hc == 0), stop=(hc == NH - 1),
                        )
                    yst = sm_pool.tile([128, 512], f32, tag="yst",
                                       name=f"yst_{tt}_{og}")
                    nc.vector.tensor_copy(yst[:], ps[:])
                    nc.sync.dma_start(
                        y_d[tt * 128:(tt + 1) * 128, og * 512:(og + 1) * 512],
                        yst[:])

                dq = deque()
                for qg in range(TGC):
                    for h in range(NH):
                        emit_cgroup(qg, h)
                        for _ in range(2):
                            if dq:
                                emit_dblock(*dq.popleft())
                    for tt in range(qg * 4, qg * 4 + 4):
                        for og in range(ODG):
                            dq.append((tt, og))
                while dq:
                    emit_dblock(*dq.popleft())

    nc.compile()
    return nc


def _augment(mat, bias_row, pad_to):
    """Append [bias_row; zeros] below mat so it has pad_to rows."""
    extra = np.zeros((pad_to - mat.shape[0], mat.shape[1]), np.float32)
    extra[0] = bias_row
    return np.concatenate([mat, extra], axis=0)


_NC_CACHE = {}


def _get_nc(bias):
    if bias not in _NC_CACHE:
        _NC_CACHE[bias] = build_nc(bias=bias)
    return _NC_CACHE[bias]


def kernel(x, Wq, bq, Wk, bk, Wv, bv, Wp, bp):
    global LAST_RESULT
    x = np.ascontiguousarray(np.asarray(x, np.float32))
    Wq, bq = np.asarray(Wq, np.float32), np.asarray(bq, np.float32)
    Wk, bk = np.asarray(Wk, np.float32), np.asarray(bk, np.float32)
    Wv, bv = np.asarray(Wv, np.float32), np.asarray(bv, np.float32)
    Wp, bp = np.asarray(Wp, np.float32), np.asarray(bp, np.float32)

    B, T, C = x.shape
    assert (B, T, C) == (4, 2048, 2048), (B, T, C)
    D = 1024  # head-group width: 8 heads per core
    bias = bool(np.any(bq) or np.any(bk) or np.any(bv))
    nc = _get_nc(bias)

    kk = np.arange(128)[:, None]
    qq = np.arange(128)[None, :]
    tri = (kk <= qq).astype(BF)
    ones = np.ones((128, 1), BF)
    Ep = C + 128 if bias else C

    in_maps = []
    for c in range(N_CORES):
        b, g = c // 2, c % 2
        xt = x[b].T
        wq_g = Wq[:, g * D:(g + 1) * D]
        wk_g = Wk[:, g * D:(g + 1) * D]
        wv_g = Wv[:, g * D:(g + 1) * D]
        if bias:
            xt = _augment(xt, np.ones(T, np.float32), Ep)
            wq_g = _augment(wq_g, bq[g * D:(g + 1) * D], Ep)
            wk_g = _augment(wk_g, bk[g * D:(g + 1) * D], Ep)
            wv_g = _augment(wv_g, bv[g * D:(g + 1) * D], Ep)
        in_maps.append({
            "xT": np.ascontiguousarray(xt.astype(BF)),
            "wq": np.ascontiguousarray(wq_g.astype(BF)),
            "wk": np.ascontiguousarray(wk_g.astype(BF)),
            "wv": np.ascontiguousarray(wv_g.astype(BF)),
            "wp": np.ascontiguousarray(Wp[g * D:(g + 1) * D, :].astype(BF)),
            "tri": tri,
            "ones": ones,
        })

    trace = bool(os.environ.get("MHA_TRACE"))
    res = run_bass_kernel_spmd(nc, in_maps, core_ids=list(range(N_CORES)),
                               trace=trace)
    LAST_RESULT = res

    out = np.empty((B, T, C), np.float32)
    for b in range(B):
        out[b] = res.results[2 * b]["y"] + res.results[2 * b + 1]["y"]
    out += bp[None, None, :]
    return out


# revision 9
# speedup vs baseline: 1.1388x; 1.0163x over previous
"""TRN2 Bass kernel for 16-head causal MHA (B=4, T=2048, C=2048), fp32 in/out.

Sharding: 8 cores = 4 batches x 2 head-groups (8 heads each).  Each core
computes q/k/v projections for its head group on its batch (tensor-parallel
column split of Wq/Wk/Wv), causal attention in the S^T layout, and a partial
output projection with the row slice of Wp.  The two head-group partials per
batch are summed on the host, plus the output bias.

Design (v2, bf16 datapath):
- All operand data (x^T, Wq/Wk/Wv/Wp, q/k/v, p, atn) is bf16; every matmul
  accumulates in fp32 PSUM.  bf16 stationaries get FWL (fast weight load,
  ~53ns/128-col) so LDWEIGHTS hides fully under the 512-col moving pass.
- q^T/k^T ([d,t] layout) and v ([t,d] layout) stay resident in SBUF (12 MB)
  -- no DRAM spill roundtrips between projection and attention.
- Softmax in the S^T layout: scores^T [k,q] per 128-k-chunk; exp on the
  Scalar engine batched over two PSUM banks ([128,1024]) to amortize the
  ~352-cycle ACTIVATE overhead; causal masking is multiplicative (one
  [128,128] lower-triangle bf16 tile on DVE, only on the 4 diagonal 128x128
  sub-blocks per (head, q-group)); strictly-above-diagonal work is skipped,
  and the diagonal blocks are column-trimmed (packed variable-width scores /
  attn / dsum matmuls).
- Softmax denominator via ones-matmul ([1,512]-out, accumulated in PSUM over
  k-chunks); reciprocal is reshaped through a DRAM roundtrip to [128,4] so
  the DVE reciprocal takes ~85ns instead of 3.3us on one lane; the recip row
  is broadcast back via a partition-stride-0 DMA read and applied by GPSIMD.
- Output projection (Wp row-slice) is interleaved per q-group into the
  attention stream one q-group behind, so its PE work fills attention's
  ACT-bound stretches; y tiles DMA out on the Vector queue as they finish.
"""
import math
import os
from collections import deque

import ml_dtypes
import numpy as np

import concourse.bass as bass
import concourse.tile as tile
from concourse import bacc, mybir
from concourse.bass_utils import run_bass_kernel_spmd

f32 = mybir.dt.float32
bf16 = mybir.dt.bfloat16
AF = mybir.ActivationFunctionType
BF = ml_dtypes.bfloat16

N_CORES = 8
HD = 128                      # head dim

# results of the last run_bass_kernel_spmd call (for test harness profiling)
LAST_RESULT = None


def build_nc(T=2048, E=2048, D=1024, NOD=2048, TG=512, bias=False,
             num_devices=N_CORES):
    """Build + compile the per-core Bass program.

    T: sequence length; E: embedding (contraction) dim; D: this core's head
    slice width (NH = D/128 heads); NOD: output projection width; TG: q-group
    width for attention; bias: if True, inputs carry one extra 128-row chunk
    holding [bias; zeros] against an xT with a ones row.
    """
    NH = D // HD              # heads per core
    EC = E // 128 + (1 if bias else 0)
    TC = T // 128             # 128-row tiles along T
    TGC = T // TG             # q-groups
    ODG = NOD // 512          # out-proj column groups
    VN = 512                  # v-projection moving width
    NVS = D // VN
    NSLOT = NH * TGC
    scale = 1.0 / math.sqrt(HD)

    nc = bacc.Bacc("TRN2", target_bir_lowering=False, debug=False,
                   num_devices=num_devices)

    xT_d = nc.dram_tensor("xT", [EC * 128, T], bf16, kind="ExternalInput")
    wq_d = nc.dram_tensor("wq", [EC * 128, D], bf16, kind="ExternalInput")
    wk_d = nc.dram_tensor("wk", [EC * 128, D], bf16, kind="ExternalInput")
    wv_d = nc.dram_tensor("wv", [EC * 128, D], bf16, kind="ExternalInput")
    wp_d = nc.dram_tensor("wp", [D, NOD], bf16, kind="ExternalInput")
    tri_d = nc.dram_tensor("tri", [128, 128], bf16, kind="ExternalInput")
    ones_d = nc.dram_tensor("ones", [128, 1], bf16, kind="ExternalInput")
    y_d = nc.dram_tensor("y", [T, NOD], f32, kind="ExternalOutput")

    dsraw = nc.dram_tensor("dsraw", [NSLOT, TG], f32, kind="Internal")
    dsrec = nc.dram_tensor("dsrec", [NSLOT, TG], f32, kind="Internal")
    dsraw_ap = dsraw.ap()
    dsrec_ap = dsrec.ap()

    with tile.TileContext(nc) as tc:
        with tc.tile_pool(name="persist", bufs=1) as persist:
            # q^T/k^T in [d, t] layout (head h = 128-row chunk h), v in
            # natural [t, d] layout ([t%128, tt*D + d]).
            qt_all = persist.tile([128, NH * T], bf16)
            kt_all = persist.tile([128, NH * T], bf16)
            v_all = persist.tile([128, TC * D], bf16)
            ones_sb = persist.tile([128, 1], bf16)
            tri_sb = persist.tile([128, 128], bf16)
            scr = persist.tile([1, 1], f32)
            nc.sync.dma_start(ones_sb[:], ones_d[:])
            nc.sync.dma_start(tri_sb[:], tri_d[:])
            # dummy exp: pulls the ~2.7us ACT table load off phase C's
            # critical path (runs during the projection phase).
            nc.scalar.activation(scr[:], ones_sb[0:1, 0:1], AF.Exp, scale=1.0)

            # ---------------- phase A+B: q/k/v projections ----------------
            with (
                tc.tile_pool(name="xt", bufs=1) as xt_pool,
                tc.tile_pool(name="wcola", bufs=3) as wcol_pool,
                tc.tile_pool(name="wvp", bufs=1) as wv_pool,
                tc.tile_pool(name="ab_psum", bufs=8, space="PSUM") as ab_psum,
            ):
                xt_sb = xt_pool.tile([128, EC * T], bf16)

                def xt_e(e):
                    return xt_sb[:, e * T:(e + 1) * T]

                dsts = (qt_all, kt_all)
                wds = (wq_d, wk_d)
                wpairs = [(w_i, dc) for w_i in range(2) for dc in range(D // 128)]

                def load_wcol(w_i, dc):
                    wcol = wcol_pool.tile([128, EC * 128], bf16, tag="wcol",
                                          name=f"wcol_{w_i}_{dc}")
                    nc.sync.dma_start(
                        wcol.rearrange("p (ec d) -> p ec d", ec=EC),
                        wds[w_i].rearrange("(ec p) d -> p ec d", p=128)[
                            :, :, dc * 128:(dc + 1) * 128],
                    )
                    return wcol

                # first two weight columns BEFORE the xT chunks so the first
                # wave's matmuls start as soon as xT chunk 0 lands.
                wcol_q = [load_wcol(*wpairs[0])]
                nc.sync.dma_start(xt_sb[:, 0:T], xT_d[0:128, :])
                wcol_q.append(load_wcol(*wpairs[1]))
                # split the xT load across two DMA queues so chunk arrival
                # (~1.4us/chunk on one queue) keeps up with the first wave
                # pair's consumption
                for e in range(1, EC):
                    eng = nc.sync if e % 2 == 0 else nc.scalar
                    eng.dma_start(
                        xt_sb[:, e * T:(e + 1) * T],
                        xT_d[e * 128:(e + 1) * 128, :],
                    )
                wvgs = []
                for dg in range(NVS):
                    wvg = wv_pool.tile([128, EC * VN], bf16, tag=f"wvg{dg}",
                                       name=f"wvg_{dg}")
                    nc.sync.dma_start(
                        wvg.rearrange("p (ec d) -> p ec d", ec=EC),
                        wv_d.rearrange("(ec p) d -> p ec d", p=128)[
                            :, :, dg * VN:(dg + 1) * VN],
                    )
                    wvgs.append(wvg)

                # Q^T / K^T: one (weight, d-chunk) per wave of 4 PSUM banks,
                # e-major inside the wave; 8 banks = two waves in flight.
                # PSUM results copy straight into the resident qt/kt tiles.
                # The first TWO waves are e-interleaved so each arriving xT
                # chunk feeds 8 matmuls (~1.7us) instead of 4 (~0.85us),
                # matching the chunk DMA arrival rate.
                ngrp = T // TG

                def qk_wave_tiles(w_i, dc):
                    return [ab_psum.tile([128, TG], f32, tag="abps",
                                         name=f"abps_{w_i}_{dc}_{tg}")
                            for tg in range(ngrp)]

                def qk_wave_mms(pss, wcol, e):
                    for tg in range(ngrp):
                        nc.tensor.matmul(
                            pss[tg][:],
                            wcol[:, e * 128:(e + 1) * 128],
                            xt_e(e)[:, tg * TG:(tg + 1) * TG],
                            start=(e == 0), stop=(e == EC - 1),
                        )

                def qk_wave_copies(pss, w_i, dc):
                    for tg in range(ngrp):
                        nc.scalar.copy(
                            dsts[w_i][:, dc * T + tg * TG:dc * T + (tg + 1) * TG],
                            pss[tg][:])

                wcol_q.append(load_wcol(*wpairs[2]))
                wcol_q.append(load_wcol(*wpairs[3]))
                pss0 = qk_wave_tiles(*wpairs[0])
                pss1 = qk_wave_tiles(*wpairs[1])
                wcol0, wcol1 = wcol_q.pop(0), wcol_q.pop(0)
                for e in range(EC):
                    qk_wave_mms(pss0, wcol0, e)
                    qk_wave_mms(pss1, wcol1, e)
                qk_wave_copies(pss0, *wpairs[0])
                qk_wave_copies(pss1, *wpairs[1])
                for wi in range(2, len(wpairs)):
                    w_i, dc = wpairs[wi]
                    wcol = wcol_q.pop(0)
                    if wi + 2 < len(wpairs):
                        wcol_q.append(load_wcol(*wpairs[wi + 2]))
                    pss = qk_wave_tiles(w_i, dc)
                    for e in range(EC):
                        qk_wave_mms(pss, wcol, e)
                    qk_wave_copies(pss, w_i, dc)

                # V in natural [t, d] layout at full 512 moving width.
                for tt in range(TC):
                    for dg in range(NVS):
                        ps = ab_psum.tile([128, VN], f32, tag="abps",
                                          name=f"vps_{tt}_{dg}")
                        for e in range(EC):
                            nc.tensor.matmul(
                                ps[:],
                                xt_e(e)[:, tt * 128:(tt + 1) * 128],
                                wvgs[dg][:, e * VN:(e + 1) * VN],
                                start=(e == 0), stop=(e == EC - 1),
                            )
                        nc.scalar.copy(
                            v_all[:, tt * D + dg * VN:tt * D + (dg + 1) * VN],
                            ps[:])

            # ---------------- phase C+D: attention + out-proj ----------------
            with (
                tc.tile_pool(name="cd", bufs=1) as cd_pool,
                tc.tile_pool(name="pt", bufs=3) as pt_pool,
                tc.tile_pool(name="sm", bufs=2) as sm_pool,
                tc.tile_pool(name="s_psum", bufs=2, space="PSUM") as s_psum,
                tc.tile_pool(name="a_psum", bufs=1, space="PSUM") as a_psum,
                tc.tile_pool(name="d_psum", bufs=1, space="PSUM") as d_psum,
                tc.tile_pool(name="y_psum", bufs=2, space="PSUM") as y_psum,
            ):
                atn_all = cd_pool.tile([128, NH * T], bf16)
                wp_sb = cd_pool.tile([128, NH * ODG * 512], bf16)
                # wp load issued on the Scalar queue so the Sync queue stays
                # dedicated to the low-latency reciprocal DMA chains.
                nc.scalar.dma_start(
                    wp_sb.rearrange("p (hc og o) -> p hc og o", hc=NH, og=ODG),
                    wp_d.rearrange("(hc p) (og o) -> p hc og o", p=128, o=512),
                )
                dsum_t = d_psum.tile([1, TG], f32)

                def emit_cgroup(qg, h):
                    qbase = qg * TG
                    npairs = 2 * (qg + 1)
                    nk = 4 * (qg + 1)
                    kc0 = qg * 4           # first diagonal k-chunk

                    def pair_desc(p):
                        # [(kc, soff, width, qoff)], exp width
                        if p == npairs - 2:
                            return [(kc0, 0, 512, 0),
                                    (kc0 + 1, 512, 384, 128)], 896
                        if p == npairs - 1:
                            return [(kc0 + 2, 0, 256, 256),
                                    (kc0 + 3, 256, 128, 384)], 384
                        return [(2 * p, 0, 512, 0),
                                (2 * p + 1, 512, 512, 0)], 1024

                    pts = [None] * npairs

                    def emit_av(p):
                        # attn + dsum matmuls for pair p (after its exp/mask)
                        parts, _ = pair_desc(p)
                        p_t = pts[p]
                        for (kc, soff, w, qoff) in parts:
                            nc.tensor.matmul(
                                atn_ps[:, qoff:qoff + w],
                                v_all[:, kc * D + h * HD:kc * D + (h + 1) * HD],
                                p_t[:, soff:soff + w],
                                start=(kc == 0), stop=(kc == nk - 1),
                            )
                        for (kc, soff, w, qoff) in parts:
                            nc.tensor.matmul(
                                dsum_t[0:1, qoff:qoff + w],
                                ones_sb[:],
                                p_t[:, soff:soff + w],
                                start=(kc == 0), stop=(kc == nk - 1),
                            )

                    atn_ps = a_psum.tile([128, TG], f32, tag="atn",
                                         name=f"atn_{qg}_{h}")
                    for p in range(npairs):
                        parts, expw = pair_desc(p)
                        s_pair = s_psum.tile([128, 2 * TG], f32, tag="sp",
                                             name=f"sp_{qg}_{h}_{p}")
                        # pair B packs both score blocks into one PSUM bank:
                        # exactly one start (bank pending-zero mark) and one
                        # stop per bank, with the second block overwriting
                        # its own (still-pending) byte range.
                        packed = p == npairs - 1
                        for pi, (kc, soff, w, qoff) in enumerate(parts):
                            nc.tensor.matmul(
                                s_pair[:, soff:soff + w],
                                kt_all[:, h * T + kc * 128:h * T + (kc + 1) * 128],
                                qt_all[:, h * T + qbase + qoff:h * T + qbase + 512],
                                start=(not packed or pi == 0),
                                stop=(not packed or pi == len(parts) - 1),
                            )
                        p_t = pt_pool.tile([128, 2 * TG], bf16, tag="pt",
                                           name=f"pt_{qg}_{h}_{p}")
                        pts[p] = p_t
                        nc.scalar.activation(p_t[:, 0:expw], s_pair[:, 0:expw],
                                             AF.Exp, scale=scale)
                        if p >= npairs - 2:
                            # multiplicative causal mask on the two 128-wide
                            # partial-triangle sections of this pair
                            for (kc, soff, w, qoff) in parts:
                                nc.vector.tensor_mul(
                                    p_t[:, soff:soff + 128],
                                    p_t[:, soff:soff + 128],
                                    tri_sb[:])
                        if p > 0:
                            emit_av(p - 1)
                    emit_av(npairs - 1)

                    # normalization: dsum -> DRAM -> [128,4] recip -> DRAM ->
                    # [128,TG] broadcast -> gpsimd multiply into atn_all
                    slot = qg * NH + h
                    atn_u = sm_pool.tile([128, TG], f32, tag="atnu",
                                         name=f"atnu_{slot}")
                    nc.vector.tensor_copy(atn_u[:], atn_ps[:])
                    ds_sb = sm_pool.tile([1, TG], f32, tag="dssb",
                                         name=f"dssb_{slot}")
                    nc.vector.tensor_copy(ds_sb[:], dsum_t[:])
                    nc.sync.dma_start(dsraw[slot:slot + 1, :], ds_sb[:])
                    dsr = sm_pool.tile([128, 4], f32, tag="dsr",
                                       name=f"dsr_{slot}")
                    nc.sync.dma_start(
                        dsr[:],
                        bass.AP(tensor=dsraw_ap.tensor, offset=slot * TG,
                                ap=[[4, 128], [1, 4]]))
                    rr = sm_pool.tile([128, 4], f32, tag="rr",
                                      name=f"rr_{slot}")
                    nc.vector.reciprocal(rr[:], dsr[:])
                    nc.sync.dma_start(
                        bass.AP(tensor=dsrec_ap.tensor, offset=slot * TG,
                                ap=[[4, 128], [1, 4]]),
                        rr[:])
                    recipB = sm_pool.tile([128, TG], f32, tag="rB",
                                          name=f"rB_{slot}")
                    nc.gpsimd.dma_start(
                        out=recipB[:],
                        in_=bass.AP(tensor=dsrec_ap.tensor, offset=slot * TG,
                                    ap=[[0, 128], [1, TG]]))
                    nc.gpsimd.tensor_mul(
                        atn_all[:, h * T + qbase:h * T + qbase + TG],
                        atn_u[:], recipB[:])

                def emit_dblock(tt, og):
                    ps = y_psum.tile([128, 512], f32, tag="yps",
                                     name=f"yps_{tt}_{og}")
                    for hc in range(NH):
                        nc.tensor.matmul(
                            ps[:],
                            atn_all[:, hc * T + tt * 128:hc * T + (tt + 1) * 128],
                            wp_sb[:, (hc * ODG + og) * 512:(hc * ODG + og + 1) * 512],
                            start=(hc == 0), stop=(hc == NH - 1),
                        )
                    yst = sm_pool.tile([128, 512], f32, tag="yst",
                                       name=f"yst_{tt}_{og}")
                    nc.vector.tensor_copy(yst[:], ps[:])
                    nc.sync.dma_start(
                        y_d[tt * 128:(tt + 1) * 128, og * 512:(og + 1) * 512],
                        yst[:])

                # q-groups in DESCENDING nk order: the deep-pipelined qg=3
                # runs first (fewest exp-wait stalls with nothing to fill
                # them); each later (shallower) group gets the previous
                # group's out-projection blocks interleaved as PE filler.
                dq = deque()
                for qg in reversed(range(TGC)):
                    for h in range(NH):
                        emit_cgroup(qg, h)
                        for _ in range(2):
                            if dq:
                                emit_dblock(*dq.popleft())
                    for tt in range(qg * 4, qg * 4 + 4):
                        for og in range(ODG):
                            dq.append((tt, og))
                while dq:
                    emit_dblock(*dq.popleft())

    nc.compile()
    return nc


def _augment(mat, bias_row, pad_to):
    """Append [bias_row; zeros] below mat so it has pad_to rows."""
    extra = np.zeros((pad_to - mat.shape[0], mat.shape[1]), np.float32)
    extra[0] = bias_row
    return np.concatenate([mat, extra], axis=0)


_NC_CACHE = {}


def _get_nc(bias):
    if bias not in _NC_CACHE:
        _NC_CACHE[bias] = build_nc(bias=bias)
    return _NC_CACHE[bias]


def kernel(x, Wq, bq, Wk, bk, Wv, bv, Wp, bp):
    global LAST_RESULT
    x = np.ascontiguousarray(np.asarray(x, np.float32))
    Wq, bq = np.asarray(Wq, np.float32), np.asarray(bq, np.float32)
    Wk, bk = np.asarray(Wk, np.float32), np.asarray(bk, np.float32)
    Wv, bv = np.asarray(Wv, np.float32), np.asarray(bv, np.float32)
    Wp, bp = np.asarray(Wp, np.float32), np.asarray(bp, np.float32)

    B, T, C = x.shape
    assert (B, T, C) == (4, 2048, 2048), (B, T, C)
    D = 1024  # head-group width: 8 heads per core
    bias = bool(np.any(bq) or np.any(bk) or np.any(bv))
    nc = _get_nc(bias)

    kk = np.arange(128)[:, None]
    qq = np.arange(128)[None, :]
    tri = (kk <= qq).astype(BF)
    ones = np.ones((128, 1), BF)
    Ep = C + 128 if bias else C

    in_maps = []
    for c in range(N_CORES):
        b, g = c // 2, c % 2
        xt = x[b].T
        wq_g = Wq[:, g * D:(g + 1) * D]
        wk_g = Wk[:, g * D:(g + 1) * D]
        wv_g = Wv[:, g * D:(g + 1) * D]
        if bias:
            xt = _augment(xt, np.ones(T, np.float32), Ep)
            wq_g = _augment(wq_g, bq[g * D:(g + 1) * D], Ep)
            wk_g = _augment(wk_g, bk[g * D:(g + 1) * D], Ep)
            wv_g = _augment(wv_g, bv[g * D:(g + 1) * D], Ep)
        in_maps.append({
            "xT": np.ascontiguousarray(xt.astype(BF)),
            "wq": np.ascontiguousarray(wq_g.astype(BF)),
            "wk": np.ascontiguousarray(wk_g.astype(BF)),
            "wv": np.ascontiguousarray(wv_g.astype(BF)),
            "wp": np.ascontiguousarray(Wp[g * D:(g + 1) * D, :].astype(BF)),
            "tri": tri,
            "ones": ones,
        })

    trace = bool(os.environ.get("MHA_TRACE"))
    res = run_bass_kernel_spmd(nc, in_maps, core_ids=list(range(N_CORES)),
                               trace=trace)
    LAST_RESULT = res

    out = np.empty((B, T, C), np.float32)
    for b in range(B):
        out[b] = res.results[2 * b]["y"] + res.results[2 * b + 1]["y"]
    out += bp[None, None, :]
    return out


# revision 19
# speedup vs baseline: 1.1982x; 1.0522x over previous
"""TRN2 Bass kernel for 16-head causal MHA (B=4, T=2048, C=2048), fp32 in/out.

Sharding: 8 cores = 4 batches x 2 head-groups (8 heads each).  Each core
computes q/k/v projections for its head group on its batch (tensor-parallel
column split of Wq/Wk/Wv), causal attention in the S^T layout, and a partial
output projection with the row slice of Wp.  The two head-group partials per
batch are summed on the host, plus the output bias.

Design (v2, bf16 datapath):
- All operand data (x^T, Wq/Wk/Wv/Wp, q/k/v, p, atn) is bf16; every matmul
  accumulates in fp32 PSUM.  bf16 stationaries get FWL (fast weight load,
  ~53ns/128-col) so LDWEIGHTS hides fully under the 512-col moving pass.
- q^T/k^T ([d,t] layout) and v ([t,d] layout) stay resident in SBUF (12 MB)
  -- no DRAM spill roundtrips between projection and attention.
- Softmax in the S^T layout: scores^T [k,q] per 128-k-chunk; exp on the
  Scalar engine batched over two PSUM banks ([128,1024]) to amortize the
  ~352-cycle ACTIVATE overhead; causal masking is multiplicative (one
  [128,128] lower-triangle bf16 tile on DVE, only on the 4 diagonal 128x128
  sub-blocks per (head, q-group)); strictly-above-diagonal work is skipped,
  and the diagonal blocks are column-trimmed (packed variable-width scores /
  attn / dsum matmuls).
- Softmax denominator via ones-matmul ([1,512]-out, accumulated in PSUM over
  k-chunks); reciprocal is reshaped through a DRAM roundtrip to [128,4] so
  the DVE reciprocal takes ~85ns instead of 3.3us on one lane; the recip row
  is broadcast back via a partition-stride-0 DMA read and applied by GPSIMD.
- Output projection (Wp row-slice) is interleaved per q-group into the
  attention stream one q-group behind, so its PE work fills attention's
  ACT-bound stretches; y tiles DMA out on the Vector queue as they finish.
"""
import math
import os
from collections import deque

import ml_dtypes
import numpy as np

import concourse.bass as bass
import concourse.tile as tile
from concourse import bacc, mybir
from concourse.bass_utils import run_bass_kernel_spmd

f32 = mybir.dt.float32
bf16 = mybir.dt.bfloat16
AF = mybir.ActivationFunctionType
BF = ml_dtypes.bfloat16

N_CORES = 8
HD = 128                      # head dim

# results of the last run_bass_kernel_spmd call (for test harness profiling)
LAST_RESULT = None


def build_nc(T=2048, E=2048, D=1024, NOD=2048, TG=512, bias=False,
             num_devices=N_CORES):
    """Build + compile the per-core Bass program.

    T: sequence length; E: embedding (contraction) dim; D: this core's head
    slice width (NH = D/128 heads); NOD: output projection width; TG: q-group
    width for attention; bias: if True, inputs carry one extra 128-row chunk
    holding [bias; zeros] against an xT with a ones row.
    """
    NH = D // HD              # heads per core
    EC = E // 128 + (1 if bias else 0)
    TC = T // 128             # 128-row tiles along T
    TGC = T // TG             # q-groups
    ODG = NOD // 512          # out-proj column groups
    VN = 512                  # v-projection moving width
    NVS = D // VN
    NSLOT = NH * TGC
    scale = 1.0 / math.sqrt(HD)

    nc = bacc.Bacc("TRN2", target_bir_lowering=False, debug=False,
                   num_devices=num_devices)

    # wq/wk/wv come pre-swizzled from the host so each per-wave slice is
    # partition-contiguous in DRAM (4KB DMA packets instead of 256B).
    xT_d = nc.dram_tensor("xT", [EC * 128, T], bf16, kind="ExternalInput")
    wq_d = nc.dram_tensor("wq", [D // 128, 128, EC * 128], bf16,
                          kind="ExternalInput")
    wk_d = nc.dram_tensor("wk", [D // 128, 128, EC * 128], bf16,
                          kind="ExternalInput")
    wv_d = nc.dram_tensor("wv", [NVS, 128, EC * VN], bf16,
                          kind="ExternalInput")
    wp_d = nc.dram_tensor("wp", [D, NOD], bf16, kind="ExternalInput")
    tri_d = nc.dram_tensor("tri", [128, 128], bf16, kind="ExternalInput")
    ones_d = nc.dram_tensor("ones", [128, 1], bf16, kind="ExternalInput")
    y_d = nc.dram_tensor("y", [T, NOD], f32, kind="ExternalOutput")

    dsraw = nc.dram_tensor("dsraw", [NSLOT, TG], f32, kind="Internal")
    dsrec = nc.dram_tensor("dsrec", [NSLOT, TG], f32, kind="Internal")
    dsraw_ap = dsraw.ap()
    dsrec_ap = dsrec.ap()

    with tile.TileContext(nc) as tc:
        with tc.tile_pool(name="persist", bufs=1) as persist:
            # q^T/k^T in [d, t] layout (head h = 128-row chunk h), v in
            # natural [t, d] layout ([t%128, tt*D + d]).
            qt_all = persist.tile([128, NH * T], bf16)
            kt_all = persist.tile([128, NH * T], bf16)
            v_all = persist.tile([128, TC * D], bf16)
            ones_sb = persist.tile([128, 1], bf16)
            tri_sb = persist.tile([128, 128], bf16)
            scr = persist.tile([1, 1], f32)
            nc.sync.dma_start(ones_sb[:], ones_d[:])
            nc.sync.dma_start(tri_sb[:], tri_d[:])

            # ---------------- phase A+B: q/k/v projections ----------------
            with (
                tc.tile_pool(name="xt", bufs=1) as xt_pool,
                tc.tile_pool(name="wcola", bufs=3) as wcol_pool,
                tc.tile_pool(name="wvp", bufs=1) as wv_pool,
                tc.tile_pool(name="ab_psum", bufs=8, space="PSUM") as ab_psum,
            ):
                xt_sb = xt_pool.tile([128, EC * T], bf16)

                def xt_e(e):
                    return xt_sb[:, e * T:(e + 1) * T]

                dsts = (qt_all, kt_all)
                wds = (wq_d, wk_d)
                wpairs = [(w_i, dc) for w_i in range(2) for dc in range(D // 128)]

                def load_wcol(w_i, dc):
                    wcol = wcol_pool.tile([128, EC * 128], bf16, tag="wcol",
                                          name=f"wcol_{w_i}_{dc}")
                    nc.sync.dma_start(wcol[:], wds[w_i][dc])
                    return wcol

                # weights + consts on the Sync queue, the full xT stream on
                # the Scalar queue: chunk arrival (~1.5us each) then matches
                # the first wave pair's e-consumption (~1.7us per chunk).
                wcol_q = [load_wcol(*wpairs[0]), load_wcol(*wpairs[1])]
                for e in range(EC):
                    nc.scalar.dma_start(
                        xt_sb[:, e * T:(e + 1) * T],
                        xT_d[e * 128:(e + 1) * 128, :],
                    )
                # dummy exp (after the xT triggers so it doesn't block them):
                # pulls the ~2.7us ACT table load off phase C's critical path.
                nc.scalar.activation(scr[:], ones_sb[0:1, 0:1], AF.Exp,
                                     scale=1.0)
                wvgs = []
                for dg in range(NVS):
                    wvg = wv_pool.tile([128, EC * VN], bf16, tag=f"wvg{dg}",
                                       name=f"wvg_{dg}")
                    nc.sync.dma_start(wvg[:], wv_d[dg])
                    wvgs.append(wvg)

                # Q^T / K^T: one (weight, d-chunk) per wave of 4 PSUM banks,
                # e-major inside the wave; 8 banks = two waves in flight.
                # PSUM results copy straight into the resident qt/kt tiles.
                # The first TWO waves are e-interleaved so each arriving xT
                # chunk feeds 8 matmuls (~1.7us) instead of 4 (~0.85us),
                # matching the chunk DMA arrival rate.
                ngrp = T // TG

                def qk_wave_tiles(w_i, dc):
                    return [ab_psum.tile([128, TG], f32, tag="abps",
                                         name=f"abps_{w_i}_{dc}_{tg}")
                            for tg in range(ngrp)]

                def qk_wave_mms(pss, wcol, e):
                    for tg in range(ngrp):
                        nc.tensor.matmul(
                            pss[tg][:],
                            wcol[:, e * 128:(e + 1) * 128],
                            xt_e(e)[:, tg * TG:(tg + 1) * TG],
                            start=(e == 0), stop=(e == EC - 1),
                        )

                def qk_wave_copies(pss, w_i, dc):
                    for tg in range(ngrp):
                        nc.scalar.copy(
                            dsts[w_i][:, dc * T + tg * TG:dc * T + (tg + 1) * TG],
                            pss[tg][:])

                wcol_q.append(load_wcol(*wpairs[2]))
                wcol_q.append(load_wcol(*wpairs[3]))
                pss0 = qk_wave_tiles(*wpairs[0])
                pss1 = qk_wave_tiles(*wpairs[1])
                wcol0, wcol1 = wcol_q.pop(0), wcol_q.pop(0)
                for e in range(EC):
                    qk_wave_mms(pss0, wcol0, e)
                    qk_wave_mms(pss1, wcol1, e)
                qk_wave_copies(pss0, *wpairs[0])
                qk_wave_copies(pss1, *wpairs[1])
                for wi in range(2, len(wpairs)):
                    w_i, dc = wpairs[wi]
                    wcol = wcol_q.pop(0)
                    if wi + 2 < len(wpairs):
                        wcol_q.append(load_wcol(*wpairs[wi + 2]))
                    pss = qk_wave_tiles(w_i, dc)
                    for e in range(EC):
                        qk_wave_mms(pss, wcol, e)
                    qk_wave_copies(pss, w_i, dc)

                # V in natural [t, d] layout at full 512 moving width.
                for tt in range(TC):
                    for dg in range(NVS):
                        ps = ab_psum.tile([128, VN], f32, tag="abps",
                                          name=f"vps_{tt}_{dg}")
                        for e in range(EC):
                            nc.tensor.matmul(
                                ps[:],
                                xt_e(e)[:, tt * 128:(tt + 1) * 128],
                                wvgs[dg][:, e * VN:(e + 1) * VN],
                                start=(e == 0), stop=(e == EC - 1),
                            )
                        nc.scalar.copy(
                            v_all[:, tt * D + dg * VN:tt * D + (dg + 1) * VN],
                            ps[:])

            # ---------------- phase C+D: attention + out-proj ----------------
            with (
                tc.tile_pool(name="cd", bufs=1) as cd_pool,
                tc.tile_pool(name="pt", bufs=3) as pt_pool,
                tc.tile_pool(name="sm", bufs=2) as sm_pool,
                tc.tile_pool(name="s_psum", bufs=2, space="PSUM") as s_psum,
                tc.tile_pool(name="a_psum", bufs=1, space="PSUM") as a_psum,
                tc.tile_pool(name="d_psum", bufs=1, space="PSUM") as d_psum,
                tc.tile_pool(name="y_psum", bufs=2, space="PSUM") as y_psum,
            ):
                atn_all = cd_pool.tile([128, NH * T], bf16)
                wp_sb = cd_pool.tile([128, NH * ODG * 512], bf16)
                # wp load issued on the Scalar queue so the Sync queue stays
                # dedicated to the low-latency reciprocal DMA chains.
                nc.scalar.dma_start(
                    wp_sb.rearrange("p (hc og o) -> p hc og o", hc=NH, og=ODG),
                    wp_d.rearrange("(hc p) (og o) -> p hc og o", p=128, o=512),
                )
                dsum_t = d_psum.tile([1, TG], f32)

                def emit_cgroup(qg, h):
                    qbase = qg * TG
                    npairs = 2 * (qg + 1)
                    nk = 4 * (qg + 1)
                    kc0 = qg * 4           # first diagonal k-chunk

                    def pair_desc(p):
                        # [(kc, soff, width, qoff)], exp width
                        if p == npairs - 2:
                            return [(kc0, 0, 512, 0),
                                    (kc0 + 1, 512, 384, 128)], 896
                        if p == npairs - 1:
                            return [(kc0 + 2, 0, 256, 256),
                                    (kc0 + 3, 256, 128, 384)], 384
                        return [(2 * p, 0, 512, 0),
                                (2 * p + 1, 512, 512, 0)], 1024

                    pts = [None] * npairs
                    p_sum = pt_pool.tile([128, TG], bf16, tag="psacc",
                                         bufs=2, name=f"psacc_{qg}_{h}")

                    def emit_av(p):
                        # attn matmuls + DVE p-sum accumulation for pair p
                        # (after its exp/mask); the softmax denominator comes
                        # from p_sum via one ones-matmul per group instead of
                        # one per chunk, trading 51us of PE for 90us of DVE.
                        parts, _ = pair_desc(p)
                        p_t = pts[p]
                        for (kc, soff, w, qoff) in parts:
                            nc.tensor.matmul(
                                atn_ps[:, qoff:qoff + w],
                                v_all[:, kc * D + h * HD:kc * D + (h + 1) * HD],
                                p_t[:, soff:soff + w],
                                start=(kc == 0), stop=(kc == nk - 1),
                            )
                        for (kc, soff, w, qoff) in parts:
                            if kc == 0:
                                nc.vector.tensor_copy(p_sum[:], p_t[:, 0:TG])
                            else:
                                nc.vector.tensor_add(
                                    p_sum[:, qoff:qoff + w],
                                    p_sum[:, qoff:qoff + w],
                                    p_t[:, soff:soff + w])

                    atn_ps = a_psum.tile([128, TG], f32, tag="atn",
                                         name=f"atn_{qg}_{h}")
                    for p in range(npairs):
                        parts, expw = pair_desc(p)
                        s_pair = s_psum.tile([128, 2 * TG], f32, tag="sp",
                                             name=f"sp_{qg}_{h}_{p}")
                        # pair B packs both score blocks into one PSUM bank:
                        # exactly one start (bank pending-zero mark) and one
                        # stop per bank, with the second block overwriting
                        # its own (still-pending) byte range.
                        packed = p == npairs - 1
                        for pi, (kc, soff, w, qoff) in enumerate(parts):
                            nc.tensor.matmul(
                                s_pair[:, soff:soff + w],
                                kt_all[:, h * T + kc * 128:h * T + (kc + 1) * 128],
                                qt_all[:, h * T + qbase + qoff:h * T + qbase + 512],
                                start=(not packed or pi == 0),
                                stop=(not packed or pi == len(parts) - 1),
                            )
                        p_t = pt_pool.tile([128, 2 * TG], bf16, tag="pt",
                                           name=f"pt_{qg}_{h}_{p}")
                        pts[p] = p_t
                        nc.scalar.activation(p_t[:, 0:expw], s_pair[:, 0:expw],
                                             AF.Exp, scale=scale)
                        if p >= npairs - 2:
                            # multiplicative causal mask on the two 128-wide
                            # partial-triangle sections of this pair
                            for (kc, soff, w, qoff) in parts:
                                nc.vector.tensor_mul(
                                    p_t[:, soff:soff + 128],
                                    p_t[:, soff:soff + 128],
                                    tri_sb[:])
                        if p > 0:
                            emit_av(p - 1)
                    emit_av(npairs - 1)
                    slot = qg * NH + h
                    atn_u = sm_pool.tile([128, TG], f32, tag="atnu",
                                         name=f"atnu_{slot}")
                    nc.scalar.copy(atn_u[:], atn_ps[:])

                    def finalize():
                        # denominator ones-matmul (deferred past the
                        # interleaved out-proj blocks so the DVE p_sum chain
                        # drains off the PE critical path), then:
                        # dsum -> DRAM -> [128,4] recip -> DRAM -> [128,TG]
                        # broadcast -> gpsimd multiply into atn_all
                        nc.tensor.matmul(dsum_t[:], ones_sb[:], p_sum[:],
                                         start=True, stop=True)
                        ds_sb = sm_pool.tile([1, TG], f32, tag="dssb",
                                             name=f"dssb_{slot}")
                        nc.vector.tensor_copy(ds_sb[:], dsum_t[:])
                        nc.sync.dma_start(dsraw[slot:slot + 1, :], ds_sb[:])
                        dsr = sm_pool.tile([128, 4], f32, tag="dsr",
                                           name=f"dsr_{slot}")
                        nc.sync.dma_start(
                            dsr[:],
                            bass.AP(tensor=dsraw_ap.tensor, offset=slot * TG,
                                    ap=[[4, 128], [1, 4]]))
                        rr = sm_pool.tile([128, 4], f32, tag="rr",
                                          name=f"rr_{slot}")
                        nc.vector.reciprocal(rr[:], dsr[:])
                        nc.sync.dma_start(
                            bass.AP(tensor=dsrec_ap.tensor, offset=slot * TG,
                                    ap=[[4, 128], [1, 4]]),
                            rr[:])
                        recipB = sm_pool.tile([128, TG], f32, tag="rB",
                                              name=f"rB_{slot}")
                        nc.gpsimd.dma_start(
                            out=recipB[:],
                            in_=bass.AP(tensor=dsrec_ap.tensor,
                                        offset=slot * TG,
                                        ap=[[0, 128], [1, TG]]))
                        nc.gpsimd.tensor_mul(
                            atn_all[:, h * T + qbase:h * T + qbase + TG],
                            atn_u[:], recipB[:])

                    return finalize

                def emit_dblock(tt, og):
                    ps = y_psum.tile([128, 512], f32, tag="yps",
                                     name=f"yps_{tt}_{og}")
                    for hc in range(NH):
                        nc.tensor.matmul(
                            ps[:],
                            atn_all[:, hc * T + tt * 128:hc * T + (tt + 1) * 128],
                            wp_sb[:, (hc * ODG + og) * 512:(hc * ODG + og + 1) * 512],
                            start=(hc == 0), stop=(hc == NH - 1),
                        )
                    yst = sm_pool.tile([128, 512], f32, tag="yst",
                                       name=f"yst_{tt}_{og}")
                    nc.vector.tensor_copy(yst[:], ps[:])
                    nc.sync.dma_start(
                        y_d[tt * 128:(tt + 1) * 128, og * 512:(og + 1) * 512],
                        yst[:])

                # With the denominator work on DVE, per-pair PE (4 matmuls,
                # ~850ns) sits below per-pair ACT (~1.1us exp), so deep
                # q-groups starve the PE: ascending order runs the shallow
                # (PE-balanced) groups first and fills each later group's
                # ACT-bound stretches with the previous group's
                # out-projection blocks.
                dq = deque()
                for qg in range(TGC):
                    for h in range(NH):
                        fin = emit_cgroup(qg, h)
                        for _ in range(2):
                            if dq:
                                emit_dblock(*dq.popleft())
                        fin()
                    for tt in range(qg * 4, qg * 4 + 4):
                        for og in range(ODG):
                            dq.append((tt, og))
                while dq:
                    emit_dblock(*dq.popleft())

    nc.compile()
    return nc


def _augment(mat, bias_row, pad_to):
    """Append [bias_row; zeros] below mat so it has pad_to rows."""
    extra = np.zeros((pad_to - mat.shape[0], mat.shape[1]), np.float32)
    extra[0] = bias_row
    return np.concatenate([mat, extra], axis=0)


def _swizzle_qk(w, EC):
    """[EC*128, D] -> [D//128, 128, EC*128]: per-wave slice partition-major
    so its DMA moves in 4KB packets."""
    D = w.shape[1]
    return np.ascontiguousarray(
        w.reshape(EC, 128, D // 128, 128).transpose(2, 1, 0, 3)
        .reshape(D // 128, 128, EC * 128).astype(BF))


def _swizzle_v(w, EC, VN=512):
    """[EC*128, D] -> [D//VN, 128, EC*VN] partition-major."""
    D = w.shape[1]
    return np.ascontiguousarray(
        w.reshape(EC, 128, D // VN, VN).transpose(2, 1, 0, 3)
        .reshape(D // VN, 128, EC * VN).astype(BF))


_NC_CACHE = {}


def _get_nc(bias):
    if bias not in _NC_CACHE:
        _NC_CACHE[bias] = build_nc(bias=bias)
    return _NC_CACHE[bias]


def kernel(x, Wq, bq, Wk, bk, Wv, bv, Wp, bp):
    global LAST_RESULT
    x = np.ascontiguousarray(np.asarray(x, np.float32))
    Wq, bq = np.asarray(Wq, np.float32), np.asarray(bq, np.float32)
    Wk, bk = np.asarray(Wk, np.float32), np.asarray(bk, np.float32)
    Wv, bv = np.asarray(Wv, np.float32), np.asarray(bv, np.float32)
    Wp, bp = np.asarray(Wp, np.float32), np.asarray(bp, np.float32)

    B, T, C = x.shape
    assert (B, T, C) == (4, 2048, 2048), (B, T, C)
    D = 1024  # head-group width: 8 heads per core
    bias = bool(np.any(bq) or np.any(bk) or np.any(bv))
    nc = _get_nc(bias)

    kk = np.arange(128)[:, None]
    qq = np.arange(128)[None, :]
    tri = (kk <= qq).astype(BF)
    ones = np.ones((128, 1), BF)
    Ep = C + 128 if bias else C

    in_maps = []
    for c in range(N_CORES):
        b, g = c // 2, c % 2
        xt = x[b].T
        wq_g = Wq[:, g * D:(g + 1) * D]
        wk_g = Wk[:, g * D:(g + 1) * D]
        wv_g = Wv[:, g * D:(g + 1) * D]
        if bias:
            xt = _augment(xt, np.ones(T, np.float32), Ep)
            wq_g = _augment(wq_g, bq[g * D:(g + 1) * D], Ep)
            wk_g = _augment(wk_g, bk[g * D:(g + 1) * D], Ep)
            wv_g = _augment(wv_g, bv[g * D:(g + 1) * D], Ep)
        EC = Ep // 128
        in_maps.append({
            "xT": np.ascontiguousarray(xt.astype(BF)),
            "wq": _swizzle_qk(wq_g, EC),
            "wk": _swizzle_qk(wk_g, EC),
            "wv": _swizzle_v(wv_g, EC),
            "wp": np.ascontiguousarray(Wp[g * D:(g + 1) * D, :].astype(BF)),
            "tri": tri,
            "ones": ones,
        })

    trace = bool(os.environ.get("MHA_TRACE"))
    res = run_bass_kernel_spmd(nc, in_maps, core_ids=list(range(N_CORES)),
                               trace=trace)
    LAST_RESULT = res

    out = np.empty((B, T, C), np.float32)
    for b in range(B):
        out[b] = res.results[2 * b]["y"] + res.results[2 * b + 1]["y"]
    out += bp[None, None, :]
    return out
